# revision 24
# baseline (speedup 1.0000x reference)
"""Trainium2 Bass kernel for a pre-norm transformer block (nn_Block_74766790689102).

Strategy (8 NeuronCores, zero-communication SPMD):
  core c handles batch b=c//4, query chunk q=c%4 (512 of 2048 tokens).
  Each core redundantly computes K/V for its whole batch (attention needs all
  keys), and everything else (qkv for its chunk, attention, proj, MLP) only for
  its own 512-token chunk. Inputs are host-rotated so every core's chunk sits at
  token positions 0:512 -> one identical SPMD program for all 8 cores.

  v6: V/K/Q projections in fp8 DoubleRow (weights x16 host-scaled; descaled
  exactly via the ones-column (=16) denominator and the exp scale SCALE/256).
  Chunk-pipelined lead: per 512-token chunk, LN1 stats (DVE) -> mu/rstd
  broadcast (PE matmul into nps-tagged psum, stat transposes borrow the same
  ring) -> normalize (split DVE / GpSimd) -> K(0)/scores(0)/exp + V for that
  chunk, so the exp stream starts ~15us in and the PE in-order queue is never
  blocked by a later chunk's stats. Attention loop then streams exp-bound
  with K(j)/Q(j) production and PV(j-1) interleaved at kt granularity.
  PSUM: acc(2) + nps(2->pv 2) + scores(4) banks = 8 peak.
"""

import os
import sys
import types

import numpy as np

DIM = 1024
HEADS = 16
HD = 64
HIDDEN = 4096
T = 2048          # tokens per batch
CH = 512          # chunk tokens per core
SCALE = HD ** -0.5
EPS = 1e-5
NCT = DIM // 128  # 8 c-tiles
NTC = T // 512    # 4 token chunks
NTT = T // 128    # 16 token tiles
P = 128
W8S = 64.0        # fp8 fc1 weight scale (descaled in the gelu)
WQS = 16.0        # fp8 qkv weight scale (descaled via ones-col / exp scale)
VD = 72           # vsb inner dim: cols 0:64 = V, col 64 = WQS (denominator)

_ENV_READY = False
_PROG = None


def _setup_env():
    global _ENV_READY
    if _ENV_READY:
        return
    if "/opt/trn_rl_repo" not in sys.path:
        sys.path.insert(0, "/opt/trn_rl_repo")
    # NTFF profile hook shim (the RL container's antenv lacks axon_hooks).
    try:
        import antenv
        if "antenv.axon_hooks" not in sys.modules:
            mod = types.ModuleType("antenv.axon_hooks")
            mod._hook = None
            mod.set_axon_ntff_profile_hook = lambda h: setattr(mod, "_hook", h)
            mod.get_axon_ntff_profile_hook = lambda: mod._hook
            sys.modules["antenv.axon_hooks"] = mod
            antenv.axon_hooks = mod
        if os.environ.get("BASS_PROFILE"):
            from trn_agent_boot.trn_boot import _ntff_profile_via_ctypes
            sys.modules["antenv.axon_hooks"].set_axon_ntff_profile_hook(
                _ntff_profile_via_ctypes("/opt/axon/libaxon_pjrt.so"))
    except Exception:
        pass
    _ENV_READY = True


def _build_program():
    """Build + compile the single-core Bass program (same for all 8 cores)."""
    _setup_env()
    import concourse.bacc as bacc
    import concourse.tile as tile
    import concourse.mybir as mybir
    from concourse.masks import make_identity

    dt = mybir.dt
    AF = mybir.ActivationFunctionType
    ALU = mybir.AluOpType

    nc = bacc.Bacc("TRN2", target_bir_lowering=False, debug=False, num_devices=8)

    f32, f32r, bf16, f8 = dt.float32, dt.float32r, dt.bfloat16, dt.float8e4
    DR = mybir.MatmulPerfMode.DoubleRow

    # ---- I/O ----
    xT_d = nc.dram_tensor("xT_d", [DIM, T], bf16, kind="ExternalInput").ap()
    xtm_d = nc.dram_tensor("xtm_d", [T, DIM], bf16, kind="ExternalInput").ap()
    xres_d = nc.dram_tensor("xres_d", [CH, DIM], f32, kind="ExternalInput").ap()
    wkq_d = nc.dram_tensor("wkq_d", [NCT, P, NCT, 256], f8, kind="ExternalInput").ap()
    wvT_d = nc.dram_tensor("wvT_d", [DIM, DIM], f8, kind="ExternalInput").ap()
    # bias_d free-dim layout: [bq64(8) | bk(8) | b1pp(32) | bv(1024) | b2(1024)]
    bias_d = nc.dram_tensor("bias_d", [P, 48 + 2 * DIM], f32, kind="ExternalInput").ap()
    wp_d = nc.dram_tensor("wp_d", [P, NCT, DIM], bf16, kind="ExternalInput").ap()
    w1h_d = nc.dram_tensor("w1h_d", [HIDDEN // P, P, NCT, P], dt.float8e4, kind="ExternalInput").ap()
    w2T_d = nc.dram_tensor("w2T_d", [HIDDEN, DIM], bf16, kind="ExternalInput").ap()
    out_d = nc.dram_tensor("out_d", [CH, DIM], f32, kind="ExternalOutput").ap()

    with tile.TileContext(nc) as tc:
        with tc.tile_pool(name="const", bufs=1) as cst:
            biases = cst.tile([P, 48 + 2 * DIM], f32)
            nc.gpsimd.dma_start(biases[:], bias_d[:])
            bq64 = biases[:, 0:NCT]
            bk_pp = biases[:, NCT:2 * NCT]
            b1_pp = biases[:, 16:48]
            bv_bc = biases[:, 48:48 + DIM]
            b2_bc = biases[:, 48 + DIM:48 + 2 * DIM]

            idf = cst.tile([P, P], f32)
            make_identity(nc, idf[:])
            id_bf = cst.tile([P, P], bf16)
            nc.vector.tensor_copy(id_bf[:], idf[:])
            ones_f = cst.tile([1, P], f32)
            nc.vector.memset(ones_f[:], 1.0)
            ones128_r = cst.tile([1, P], f32r)
            nc.vector.tensor_copy(ones128_r[:], ones_f[:])
            eps_t = cst.tile([P, 1], f32)
            nc.vector.memset(eps_t[:], EPS)

            o_asm = cst.tile([P, NCT, 512], bf16)  # attention out, head-pair-major

            # ================= big pool: tensors alive through attention ======
            big_cm = tc.tile_pool(name="big", bufs=1)
            big = big_cm.__enter__()
            vsb = big.tile([P, NTT, HEADS, VD], bf16)  # V-hat token-major (x WQS)
            k_sb = big.tile([P, NCT, T], bf16)         # K feature-major (x WQS)
            q_sb = big.tile([P, NCT, 512], bf16)       # Q feature-major (x WQS)

            abc_cm = tc.tile_pool(name="abc", bufs=1)
            abc = abc_cm.__enter__()
            xn_sb = abc.tile([P, NCT, T], f8)     # normalized x^T, fp8
            wv_sb = abc.tile([P, NCT, DIM], f8)
            bvr = abc.tile([1, DIM], f32r)        # bv row for the ones-row matmul
            nc.vector.tensor_copy(bvr[:], bv_bc[0:1, :])
            onesc_f = abc.tile([P, 256], f32)
            nc.vector.memset(onesc_f[:], WQS)
            nc.vector.tensor_copy(
                vsb[:, :, :, 64:65].rearrange("p a b c -> p (a b c)"), onesc_f[:])

            # wv / wkq(0) / per-chunk x^T slices ride the scalar DMA queue
            # (stats tiles own the sync queue).
            nc.scalar.dma_start(wv_sb[:], wvT_d.rearrange("(a p) o -> p a o", p=P))

            # acc psum opens FIRST, then scores; the nps ring (stats bcast +
            # stat transposes) closes after the lead and pv reuses its banks.
            kqv_cm = tc.tile_pool(name="kqvps", bufs=2, space="PSUM")
            kqv = kqv_cm.__enter__()

            kqp_cm = tc.tile_pool(name="kqp", bufs=2)
            kqpool = kqp_cm.__enter__()
            att_cm = tc.tile_pool(name="att", bufs=2)
            att = att_cm.__enter__()
            pst_cm = tc.tile_pool(name="pst", bufs=1)
            pstp = pst_cm.__enter__()
            sps_cm = tc.tile_pool(name="spsum", bufs=2, space="PSUM")
            sps = sps_cm.__enter__()

            slots = {}

            def emit_k(j, tcn, wkq):
                kp = kqv.tile([P, 512], f32, tag="acc", name=f"kp{j}_{tcn}")
                for k in range(NCT // 2):
                    nc.tensor.matmul(kp[:], wkq[:, 2 * k:2 * k + 2, 0:P],
                                     xn_sb[:, 2 * k:2 * k + 2,
                                           tcn * 512:(tcn + 1) * 512],
                                     start=(k == 0), stop=(k == NCT // 2 - 1),
                                     perf_mode=DR)
                nc.vector.tensor_scalar(k_sb[:, j, tcn * 512:(tcn + 1) * 512],
                                        kp[:], bk_pp[:, j:j + 1], None, ALU.add)

            def emit_q(j, wkq):
                qp = kqv.tile([P, 512], f32, tag="acc", name=f"qp{j}")
                for k in range(NCT // 2):
                    nc.tensor.matmul(qp[:], wkq[:, 2 * k:2 * k + 2, P:256],
                                     xn_sb[:, 2 * k:2 * k + 2, 0:512],
                                     start=(k == 0), stop=(k == NCT // 2 - 1),
                                     perf_mode=DR)
                nc.vector.tensor_scalar(q_sb[:, j, :], qp[:], bq64[:, j:j + 1],
                                        None, ALU.add)

            def emit_v(tt):
                for oc in range(2):
                    vp = kqv.tile([P, 512], f32, tag="acc", name=f"vp{tt}_{oc}")
                    # bias via ones-row matmul; drain on Scalar (DVE stays free)
                    nc.tensor.matmul(vp[:], ones128_r[:],
                                     bvr[:, oc * 512:(oc + 1) * 512],
                                     start=True, stop=False)
                    for k in range(NCT // 2):
                        nc.tensor.matmul(vp[:],
                                         xn_sb[:, 2 * k:2 * k + 2,
                                               tt * P:(tt + 1) * P],
                                         wv_sb[:, 2 * k:2 * k + 2,
                                               oc * 512:(oc + 1) * 512],
                                         start=False, stop=(k == NCT // 2 - 1),
                                         perf_mode=DR)
                    nc.scalar.activation(
                        vsb[:, tt, oc * 8:(oc + 1) * 8, 0:64],
                        vp[:].rearrange("p (h d) -> p h d", d=64), AF.Copy)

            def emit_score(js, kt):
                sp = sps.tile([P, 2, 512], f32, tag="s", name=f"sp{js}_{kt}")
                nc.tensor.matmul(sp[:, 0, :],
                                 k_sb[0:64, js, kt * P:(kt + 1) * P],
                                 q_sb[0:64, js, :], start=True, stop=True)
                nc.tensor.matmul(sp[:, 1, :],
                                 k_sb[64:128, js, kt * P:(kt + 1) * P],
                                 q_sb[64:128, js, :], start=True, stop=True)
                slot = pstp.tile([P, 2, 512], bf16, tag="slot", bufs=16)
                nc.scalar.activation(slot[:], sp[:], AF.Exp,
                                     scale=SCALE / (WQS * WQS))
                slots[(js, kt)] = slot

            # ---------------- Lead: per-chunk stats -> norm -> K(0)/S(0)/V ----
            wkq0 = kqpool.tile([P, NCT, 256], f8, tag="wkq", name="wkq0")
            nc.scalar.dma_start(wkq0[:], wkq_d[0])

            stp_cm = tc.tile_pool(name="stp", bufs=1)
            stp = stp_cm.__enter__()
            nps_cm = tc.tile_pool(name="npsum", bufs=2, space="PSUM")
            nps = nps_cm.__enter__()

            for tcn in range(NTC):
                xbf = stp.tile([P, NCT, 512], bf16, tag="xbf", bufs=2)
                for a in range(NCT):
                    nc.scalar.dma_start(
                        xbf[:, a, :],
                        xT_d[a * P:(a + 1) * P, tcn * 512:(tcn + 1) * 512])
                muc = stp.tile([1, 512], f32r, tag="muc", bufs=1)
                rsc = stp.tile([1, 512], f32r, tag="rsc", bufs=1)
                for i, s in enumerate(range(4 * tcn, 4 * tcn + 4)):
                    xs = stp.tile([P, DIM], bf16, tag="xs", bufs=2)
                    nc.sync.dma_start(xs[:], xtm_d[s * P:(s + 1) * P, :])
                    stats = stp.tile([P, 2, 6], f32, tag="bst", bufs=2)
                    for g in range(2):
                        nc.vector.bn_stats(stats[:, g, :], xs[:, g * 512:(g + 1) * 512])
                    st = stp.tile([P, 2], f32, tag="mv", bufs=3)
                    nc.vector.bn_aggr(st[:], stats[:])
                    sdv = stp.tile([P, 1], f32, tag="sdv", bufs=2)
                    nc.scalar.activation(sdv[:], st[:, 1:2], AF.Sqrt, bias=eps_t[:])
                    nc.vector.reciprocal(st[:, 1:2], sdv[:])
                    # stat transposes borrow the nps psum ring ([1,128] slices)
                    pmu = nps.tile([P, 512], f32, tag="nps", name=f"pT{s}a")
                    nc.tensor.transpose(pmu[0:1, 0:P], st[:, 0:1], idf[:])
                    nc.vector.tensor_copy(muc[:, i * P:(i + 1) * P], pmu[0:1, 0:P])
                    prs = nps.tile([P, 512], f32, tag="nps", name=f"pT{s}b")
                    nc.tensor.transpose(prs[0:1, 0:P], st[:, 1:2], idf[:])
                    nc.vector.tensor_copy(rsc[:, i * P:(i + 1) * P], prs[0:1, 0:P])

                mb = nps.tile([P, 512], f32, tag="nps", name=f"mb{tcn}")
                nc.tensor.matmul(mb[:], ones128_r[:], muc[:], start=True, stop=True)
                rb = nps.tile([P, 512], f32, tag="nps", name=f"rb{tcn}")
                nc.tensor.matmul(rb[:], ones128_r[:], rsc[:], start=True, stop=True)
                mb_sb = stp.tile([P, 512], bf16, tag="mbsb", bufs=1)
                nc.scalar.activation(mb_sb[:], mb[:], AF.Copy)
                rb_sb = stp.tile([P, 512], bf16, tag="rbsb", bufs=1)
                nc.scalar.activation(rb_sb[:], rb[:], AF.Copy)
                sl = slice(tcn * 512, (tcn + 1) * 512)
                for ct in range(NCT):
                    eng = nc.vector if ct < 4 else nc.gpsimd
                    tmp = stp.tile([P, 512], bf16, tag=f"tmp{ct // 4}", bufs=1)
                    eng.tensor_tensor(tmp[:], xbf[:, ct, :], mb_sb[:],
                                      ALU.subtract)
                    eng.tensor_tensor(xn_sb[:, ct, sl], tmp[:], rb_sb[:],
                                      ALU.mult)
                # chunk work: K(0) for this chunk, scores(0) kt range, V tiles
                emit_k(0, tcn, wkq0)
                if tcn == 0:
                    emit_q(0, wkq0)
                for kt in range(4 * tcn, 4 * tcn + 4):
                    emit_score(0, kt)
                for tt in range(4 * tcn, 4 * tcn + 4):
                    emit_v(tt)

            nps_cm.__exit__(None, None, None)
            stp_cm.__exit__(None, None, None)

            # ------------- Attention loop (exp-paced) -------------------------
            pvs_cm = tc.tile_pool(name="pvpsum", bufs=2, space="PSUM")
            pvs = pvs_cm.__enter__()

            def tail(h, pv):
                j, hh = h // 2, h % 2
                den = att.tile([1, 512], f32, tag="den", bufs=2, name=f"den{h}")
                nc.vector.tensor_copy(den[:], pv[64:65, :])
                rcf = att.tile([1, 512], f32, tag="rcf", bufs=2, name=f"rcf{h}")
                nc.vector.reciprocal_approx_fast(rcf[:], den[:])
                rcp = att.tile([1, 512], bf16, tag="rcp", bufs=3, name=f"rcp{h}")
                nc.vector.tensor_copy(rcp[:], rcf[:])
                bc_sb = att.tile([P, 512], bf16, tag="bcsb", bufs=2, name=f"bsb{h}")
                nc.gpsimd.partition_broadcast(bc_sb[0:64, :], rcp[:], channels=64)
                o_st = att.tile([P, 512], bf16, tag="ost", bufs=4, name=f"ost{h}")
                nc.vector.tensor_tensor(o_st[0:64, :], pv[0:64, :],
                                        bc_sb[0:64, :], ALU.mult)
                nc.sync.dma_start(o_asm[hh * 64:(hh + 1) * 64, j, :],
                                  o_st[0:64, :])

            for js in range(1, NCT + 1):
                if js < NCT:
                    wkq = kqpool.tile([P, NCT, 256], f8, tag="wkq", name=f"wkq{js}")
                    nc.sync.dma_start(wkq[:], wkq_d[js])
                    for tcn in range(NTC):
                        emit_k(js, tcn, wkq)
                    emit_q(js, wkq)
                jp = js - 1
                pvt = [pvs.tile([P, 512], f32, tag="pv",
                                name=f"pv{2 * jp + hh}") for hh in range(2)]
                for kt in range(NTT):
                    for hh in range(2):
                        nc.tensor.matmul(pvt[hh][0:VD, :],
                                         vsb[:, kt, 2 * jp + hh, :],
                                         slots[(jp, kt)][:, hh, :],
                                         start=(kt == 0),
                                         stop=(kt == NTT - 1))
                    if js < NCT:
                        emit_score(js, kt)
                tail(2 * jp, pvt[0])
                tail(2 * jp + 1, pvt[1])
                for kt in range(NTT):
                    del slots[(jp, kt)]

            pvs_cm.__exit__(None, None, None)
            sps_cm.__exit__(None, None, None)
            pst_cm.__exit__(None, None, None)
            att_cm.__exit__(None, None, None)
            kqp_cm.__exit__(None, None, None)
            kqv_cm.__exit__(None, None, None)
            abc_cm.__exit__(None, None, None)

            # ------- Phase E: proj + residual + LN2 + transpose ---------------
            post_cm = tc.tile_pool(name="post", bufs=1)
            post = post_cm.__enter__()
            wp_sb = post.tile([P, NCT, DIM], bf16)
            for j in range(NCT):
                nc.sync.dma_start(wp_sb[:, j, :], wp_d[:, j, :])
            res1 = post.tile([P, NTC, DIM], f32)
            h2t = post.tile([P, NCT, 512], f8)
            with tc.tile_pool(name="prj", bufs=2) as prj, \
                 tc.tile_pool(name="st2", bufs=2) as st2, \
                 tc.tile_pool(name="pjps", bufs=2, space="PSUM") as pjs, \
                 tc.tile_pool(name="tps", bufs=2, space="PSUM") as tps:
                for ts in range(4):
                    # xres_d is host-prebiased with b_proj
                    xres = prj.tile([P, DIM], f32, tag="xres", bufs=2)
                    nc.sync.dma_start(xres[:], xres_d[ts * P:(ts + 1) * P, :])
                    for oc in range(2):
                        pj = pjs.tile([P, 512], f32, tag="pj")
                        for j in range(NCT):
                            nc.tensor.matmul(pj[:], o_asm[:, j, ts * P:(ts + 1) * P],
                                             wp_sb[:, j, oc * 512:(oc + 1) * 512],
                                             start=(j == 0), stop=(j == NCT - 1))
                        nc.vector.tensor_tensor(res1[:, ts, oc * 512:(oc + 1) * 512],
                                                pj[:], xres[:, oc * 512:(oc + 1) * 512],
                                                ALU.add)
                    # LN2 for this token tile
                    stats = st2.tile([P, 2, 6], f32, tag="bst2")
                    for g in range(2):
                        nc.vector.bn_stats(stats[:, g, :],
                                           res1[:, ts, g * 512:(g + 1) * 512])
                    mv = st2.tile([P, 2], f32, tag="mv2")
                    nc.vector.bn_aggr(mv[:], stats[:])
                    sdv = st2.tile([P, 1], f32, tag="sdv2")
                    nc.scalar.activation(sdv[:], mv[:, 1:2], AF.Sqrt, bias=eps_t[:])
                    rs2 = st2.tile([P, 1], f32, tag="rs2")
                    nc.vector.reciprocal(rs2[:], sdv[:])
                    h2 = prj.tile([P, DIM], bf16, tag="h2", bufs=2)
                    nc.vector.tensor_scalar(h2[:], res1[:, ts, :], mv[:, 0:1], rs2[:],
                                            ALU.subtract, ALU.mult)
                    for ct in range(NCT):
                        tp = tps.tile([P, P], bf16, tag="tp2")
                        nc.tensor.transpose(tp[:], h2[:, ct * P:(ct + 1) * P], id_bf[:])
                        nc.scalar.activation(h2t[:, ct, ts * P:(ts + 1) * P], tp[:],
                                             AF.Copy)

            # ---------------- Phase F: MLP ----------------
            with tc.tile_pool(name="mlp", bufs=3) as mlp, \
                 tc.tile_pool(name="h3tp", bufs=1) as h3tp, \
                 tc.tile_pool(name="f1ps", bufs=2, space="PSUM") as f1s, \
                 tc.tile_pool(name="f2ps", bufs=5, space="PSUM") as f2s:
                h3t = h3tp.tile([P, HIDDEN // P, 512], bf16)

                # fc1 + gelu -> h3t (feature-major)
                for ot in range(HIDDEN // P):
                    w1c = mlp.tile([P, NCT, P], f8, tag="w1c", bufs=3)
                    nc.sync.dma_start(w1c[:], w1h_d[ot])
                    fp = f1s.tile([P, 512], f32, tag="f1")
                    for ci in range(NCT // 2):
                        nc.tensor.matmul(fp[:], w1c[:, 2 * ci:2 * ci + 2, :],
                                         h2t[:, 2 * ci:2 * ci + 2, :],
                                         start=(ci == 0), stop=(ci == NCT // 2 - 1),
                                         perf_mode=DR)
                    nc.scalar.activation(h3t[:, ot, :], fp[:], AF.Gelu,
                                         bias=b1_pp[:, ot:ot + 1], scale=1.0 / W8S)

                # fc2 + bias + residual -> out
                for oc in range(2):
                    f2t = [f2s.tile([P, 512], f32, tag="f2", name=f"f2_{oc}_{i}")
                           for i in range(4)]
                    for ct in range(HIDDEN // P):
                        w2t = mlp.tile([P, 512], bf16, tag="w2t", bufs=4)
                        nc.sync.dma_start(
                            w2t[:], w2T_d[ct * P:(ct + 1) * P, oc * 512:(oc + 1) * 512])
                        for ts in range(4):
                            nc.tensor.matmul(f2t[ts][:], h3t[:, ct, ts * P:(ts + 1) * P],
                                             w2t[:], start=(ct == 0),
                                             stop=(ct == HIDDEN // P - 1))
                    for ts in range(4):
                        t1 = mlp.tile([P, 512], f32, tag="t12")
                        nc.vector.tensor_tensor(t1[:], f2t[ts][:],
                                                b2_bc[:, oc * 512:(oc + 1) * 512], ALU.add)
                        t2 = mlp.tile([P, 512], f32, tag="t22")
                        nc.vector.tensor_tensor(t2[:], t1[:],
                                                res1[:, ts, oc * 512:(oc + 1) * 512],
                                                ALU.add)
                        nc.sync.dma_start(
                            out_d[ts * P:(ts + 1) * P, oc * 512:(oc + 1) * 512], t2[:])

            post_cm.__exit__(None, None, None)
            big_cm.__exit__(None, None, None)

    nc.compile()
    return nc


def _get_program():
    global _PROG
    if _PROG is None:
        _PROG = _build_program()
    return _PROG


def _pack_cols(wT):
    """[C, O] -> [O//128, 128(p), C//128(k), 128(o)] so each o-tile DMA is contiguous."""
    C, O = wT.shape
    # out[ot, p, k, o] = wT[k*128+p, ot*128+o]
    return np.ascontiguousarray(
        wT.reshape(C // P, P, O // P, P).transpose(2, 1, 0, 3))


def _pack_wkq(wqT, wkT):
    """Combine K and Q o-tile packs: [8, 128, 8, 256] (K cols then Q cols)."""
    k = _pack_cols(wkT)
    q = _pack_cols(wqT)
    return np.ascontiguousarray(np.concatenate([k, q], axis=3))


def _host_prep(x, ln1_g, ln1_b, w_qkv, w_proj, b_proj, ln2_g, ln2_b,
               w_fc1, b_fc1, w_fc2, b_fc2):
    """Per-core input dicts. Pure layout/weight-folding work (no activation math)."""
    import ml_dtypes
    f = np.float32
    bf = ml_dtypes.bfloat16
    f8d = ml_dtypes.float8_e4m3fn
    x = np.asarray(x, f)
    g1 = np.asarray(ln1_g, f); b1 = np.asarray(ln1_b, f)
    g2 = np.asarray(ln2_g, f); b2 = np.asarray(ln2_b, f)
    w_qkv = np.asarray(w_qkv, f); w_proj = np.asarray(w_proj, f)
    w_fc1 = np.asarray(w_fc1, f); w_fc2 = np.asarray(w_fc2, f)
    b_proj = np.asarray(b_proj, f); b_fc1 = np.asarray(b_fc1, f)
    b_fc2 = np.asarray(b_fc2, f)

    wq, wk, wv = w_qkv[0:DIM], w_qkv[DIM:2 * DIM], w_qkv[2 * DIM:3 * DIM]
    bias = np.empty((P, 48 + 2 * DIM), f)
    bias[:, 0:NCT] = (wq @ b1).reshape(NCT, P).T * WQS
    bias[:, NCT:2 * NCT] = (wk @ b1).reshape(NCT, P).T * WQS
    bias[:, 16:48] = (b_fc1 + w_fc1 @ b2).reshape(HIDDEN // P, P).T
    bias[:, 48:48 + DIM] = np.broadcast_to(wv @ b1, (P, DIM)) * WQS
    bias[:, 48 + DIM:] = np.broadcast_to(b_fc2, (P, DIM))
    shared = {
        "wkq_d": (_pack_wkq((wq * g1[None, :]).T, (wk * g1[None, :]).T)
                  * WQS).astype(f8d),
        "wvT_d": (np.ascontiguousarray((wv * g1[None, :]).T) * WQS).astype(f8d),
        "bias_d": bias,
        "wp_d": np.ascontiguousarray(
            w_proj.T.reshape(NCT, P, DIM).transpose(1, 0, 2)).astype(bf),
        "w1h_d": (_pack_cols((w_fc1 * g2[None, :]).T) * W8S).astype(f8d),
        "w2T_d": np.ascontiguousarray(w_fc2.T).astype(bf),
    }
    in_maps = []
    for core in range(8):
        b, q = core // 4, core % 4
        xroll = np.roll(x[b], -CH * q, axis=0)
        m = dict(shared)
        m["xT_d"] = np.ascontiguousarray(xroll.T).astype(bf)
        m["xtm_d"] = np.ascontiguousarray(xroll).astype(bf)
        m["xres_d"] = np.ascontiguousarray(xroll[0:CH] + b_proj[None, :])
        in_maps.append(m)
    return in_maps


def kernel(**inputs) -> np.ndarray:
    _setup_env()
    from concourse import bass_utils

    nc = _get_program()
    in_maps = _host_prep(**inputs)
    run_kwargs = {}
    if os.environ.get("BASS_PROFILE"):
        import tempfile
        run_kwargs = dict(trace=True, tmpdir=tempfile.mkdtemp(prefix="blk_prof"))
    res = bass_utils.run_bass_kernel_spmd(nc, in_maps, core_ids=list(range(8)),
                                          **run_kwargs)
    kernel.last_result = res
    x = np.asarray(inputs["x"])
    out = np.empty((2, T, DIM), np.float32)
    for core in range(8):
        b, q = core // 4, core % 4
        out[b, CH * q:CH * (q + 1), :] = res.results[core]["out_d"]
    return out
